# revision 12
# baseline (speedup 1.0000x reference)
"""Multi-head self-attention (B=4, T=2048, C=1024, H=16 heads, causal) on 8 TRN2
NeuronCores, head-tensor-parallel.

Per core c (owning heads 2c, 2c+1 = attn feature rows [c*128,(c+1)*128)):
  1. QKV for all 4 batches first (dense PE stream keeps HAM warm and gives the
     scheduler PE filler work during the ACT-paced attention phase): qT/kT/vT
     feature-major via fp32r matmuls on host-pre-transposed xT. vT is
     PE-transposed (bf16) to natural v tiles with an appended ones column per
     head so each PV matmul also emits the softmax denominator row.
  2. Causal attention: scoresT [kv, q] via row-tiled (K=64) fp32r matmul pairs
     (both heads concurrently), both heads' scores land in one 2-bank PSUM
     tile so ONE ScalarE exp (no max subtraction needed; scores ~N(0,1))
     evacuates both as bf16. Diagonal blocks get a host-precomputed edge mask
     (DVE). PV accumulates in PSUM (bf16 operands, f32 accum).
  3. Normalization: reciprocal_approx_fast of the denominator row, K=1 matmul
     against a ones vector broadcasts it across partitions, DVE multiply.
  4. Attention runs in two passes (even q-blocks then odd q-blocks); each pass
     is followed by an AllGather of its normalized [128, 4096] half so the
     first AG hides under pass 2 and the projection of the first token half
     hides under the second AG.
  5. Output projection (fp32r) for this core's 1024-token slice, selected from
     the gathered buffers with a partition_id-based dynamic slice, + bias.
Host gathers the 8 [1024 feat, 1024 tok] slices, concatenates and transposes.
"""
import ml_dtypes
import numpy as np

import concourse.bass as bass
import concourse.tile as tile
from concourse import bacc, mybir
from concourse.bass_utils import run_bass_kernel_spmd

F32 = mybir.dt.float32
F32R = mybir.dt.float32r
BF16 = mybir.dt.bfloat16

B, T, C = 4, 2048, 1024
N_HEADS, HEAD = 16, 64
N_CORES = 8
BT = B * T
TOK_PER_CORE = BT // N_CORES    # 1024
TB = 512                        # token block (matmul moving dim)
NKT = C // 128                  # 8 contraction tiles
SCALE = HEAD ** -0.5


def build():
    nc = bacc.Bacc("TRN2", target_bir_lowering=False, debug=False, num_devices=N_CORES)

    xT = nc.dram_tensor("xT", [C, BT], F32R, kind="ExternalInput")
    wqkvT = nc.dram_tensor("wqkvT", [C, 384], F32R, kind="ExternalInput")
    wprojT = nc.dram_tensor("wprojT", [C, C], BF16, kind="ExternalInput")
    bmat = nc.dram_tensor("bmat", [128, 8], F32, kind="ExternalInput")
    masks = nc.dram_tensor("masks", [128, 4, 2 * TB], BF16, kind="ExternalInput")
    ident_in = nc.dram_tensor("ident", [128, 128], BF16, kind="ExternalInput")
    onescol_in = nc.dram_tensor("onescol", [128, 1], BF16, kind="ExternalInput")

    outT = nc.dram_tensor("outT", [C, TOK_PER_CORE], F32, kind="ExternalOutput")

    # half X = even q-blocks (local token halves 0), half Y = odd q-blocks
    qT_d = nc.dram_tensor("qT_d", [128, BT], F32R)
    rnorm_d = nc.dram_tensor("rnorm_d", [16, 2 * TB], F32)
    a2i = [nc.dram_tensor(f"a2i{h}", [N_CORES, 128, TB], BF16) for h in range(2)]
    a2o = [nc.dram_tensor(f"a2o{h}", [N_CORES, 128, TB], BF16) for h in range(2)]

    xT_r = xT.ap().rearrange("(kt p) n -> p kt n", p=128)

    with tile.TileContext(nc) as tc:
        with (
            tc.tile_pool(name="consts", bufs=1) as consts,
            tc.tile_pool(name="wp", bufs=1) as wp_pool,
            tc.tile_pool(name="xt", bufs=3) as xt_pool,
            tc.tile_pool(name="qk", bufs=4) as qk_pool,
            tc.tile_pool(name="qst", bufs=2) as qst_pool,
            tc.tile_pool(name="vnat", bufs=4) as vnat_pool,
            tc.tile_pool(name="vte", bufs=2) as vte_pool,
            tc.tile_pool(name="exp", bufs=4) as exp_pool,
            tc.tile_pool(name="evac", bufs=2) as evac_pool,
            tc.tile_pool(name="sr", bufs=3) as sr_pool,
            tc.tile_pool(name="po", bufs=2) as po_pool,
            tc.tile_pool(name="ps_a", bufs=2, space="PSUM") as ps_a,  # qkv/transp/pv/proj
            tc.tile_pool(name="ps_b", bufs=2, space="PSUM") as ps_b,  # paired score tiles
        ):
            wqkv_sb = consts.tile([128, NKT, 384], F32R)
            nc.sync.dma_start(out=wqkv_sb, in_=wqkvT.ap().rearrange("(kt p) m -> p kt m", p=128))
            ident = consts.tile([128, 128], BF16)
            nc.sync.dma_start(out=ident, in_=ident_in.ap())
            onescol = consts.tile([128, 1], BF16)
            nc.sync.dma_start(out=onescol, in_=onescol_in.ap())

            kTs, vns = [], []

            # ---- QKV projections, all batches ----
            for b in range(B):
                tok0 = b * T
                kT = qk_pool.tile([128, T], F32R, tag="kT")
                v_nat = vnat_pool.tile([128, 16, 130], BF16, tag="vnat")
                kTs.append(kT); vns.append(v_nat)

                for tb in range(T // TB):
                    col0 = tok0 + tb * TB
                    xt = xt_pool.tile([128, NKT, TB], F32R, tag="xt")
                    for kt in range(NKT):
                        nc.sync.dma_start(out=xt[:, kt, :], in_=xT_r[:, kt, col0:col0 + TB])
                    for m in range(3):  # 0=q, 1=k, 2=v (feature-major)
                        ps = ps_a.tile([128, TB], F32, tag="a")
                        for kt in range(NKT):
                            nc.tensor.matmul(
                                ps[:],
                                lhsT=wqkv_sb[:, kt, m * 128:(m + 1) * 128],
                                rhs=xt[:, kt, :],
                                start=(kt == 0),
                                stop=(kt == NKT - 1),
                            )
                        sl = slice(tb * TB, (tb + 1) * TB)
                        if m == 0:
                            qo = vte_pool.tile([128, TB], F32R, tag="qo")
                            nc.scalar.copy(qo[:], ps[:])
                            nc.sync.dma_start(out=qT_d.ap()[:, col0:col0 + TB], in_=qo[:])
                        elif m == 1:
                            nc.scalar.copy(kT[:, sl], ps[:])
                        else:
                            vte = vte_pool.tile([128, TB], BF16, tag="vte")
                            nc.scalar.copy(vte[:], ps[:])
                            for q in range(TB // 128):
                                jt = tb * 4 + q
                                ps_tr = ps_a.tile([128, 128], BF16, tag="a")
                                nc.tensor.transpose(
                                    ps_tr[:], vte[:, q * 128:(q + 1) * 128], ident[:]
                                )
                                nc.vector.tensor_copy(v_nat[:, jt, 0:64], ps_tr[:, 0:64])
                                nc.vector.tensor_copy(v_nat[:, jt, 65:129], ps_tr[:, 64:128])
                                nc.vector.tensor_copy(v_nat[:, jt, 64:65], onescol[:])
                                nc.vector.tensor_copy(v_nat[:, jt, 129:130], onescol[:])

            masks_sb = consts.tile([128, 4, 2 * TB], BF16)
            nc.sync.dma_start(out=masks_sb, in_=masks.ap())
            bmat_sb = consts.tile([128, 8], F32)
            nc.sync.dma_start(out=bmat_sb, in_=bmat.ap())

            # ---- causal attention, two passes over q-blocks ----
            def attn_block(b, ib, half):
                kT, v_nat = kTs[b], vns[b]
                njt = (ib + 1) * 4
                qt = qst_pool.tile([128, TB], F32R, tag="qst")
                qcol = b * T + ib * TB
                nc.sync.dma_start(out=qt, in_=qT_d.ap()[:, qcol:qcol + TB])
                pv = ps_a.tile([65, 2 * TB], F32, tag="a")
                pvA = pv[:, 0:TB]
                pvB = pv[:, TB:2 * TB]
                for jt in range(njt):
                    jsl = slice(jt * 128, (jt + 1) * 128)
                    s = ps_b.tile([128, 2 * TB], F32, tag="s")
                    nc.tensor.matmul(
                        s[:, 0:TB], lhsT=kT[0:64, jsl], rhs=qt[0:64, :],
                        start=True, stop=True, tile_position=(0, 0),
                    )
                    nc.tensor.matmul(
                        s[:, TB:2 * TB], lhsT=kT[64:128, jsl], rhs=qt[64:128, :],
                        start=True, stop=True, tile_position=(64, 0),
                    )
                    e = exp_pool.tile([128, 2 * TB], BF16, tag="e")
                    nc.scalar.activation(e[:], s[:], mybir.ActivationFunctionType.Exp, scale=SCALE)
                    if jt >= ib * 4:  # diagonal: causal edge mask (same for both heads)
                        with nc.allow_low_precision(reason="exact 0/1 mask on bf16 probs"):
                            nc.vector.tensor_mul(e[:], e[:], masks_sb[:, jt - ib * 4, :])
                    nc.tensor.matmul(
                        pv[0:65, 0:TB], lhsT=v_nat[:, jt, 0:65], rhs=e[:, 0:TB],
                        start=(jt == 0), stop=(jt == njt - 1),
                    )
                    nc.tensor.matmul(
                        pv[0:65, TB:2 * TB], lhsT=v_nat[:, jt, 65:130], rhs=e[:, TB:2 * TB],
                        start=(jt == 0), stop=(jt == njt - 1),
                    )
                # normalize both heads and ship to the AG input for this half
                blk = half * 8 + b * 2 + ib // 2
                o_un = evac_pool.tile([64, 2 * TB], F32, tag="oun")
                nc.vector.tensor_copy(o_un[:], pv[0:64, :])
                srow = sr_pool.tile([1, 2 * TB], F32, tag="sr")
                nc.vector.tensor_copy(srow[:], pv[64:65, :])
                r32 = sr_pool.tile([1, 2 * TB], F32, tag="sr")
                nc.vector.reciprocal_approx_fast(out=r32[:], in_=srow[:])
                nc.sync.dma_start(out=rnorm_d.ap()[blk, :], in_=r32[:])
                rb = evac_pool.tile([64, 2 * TB], F32, tag="rb")
                base = rnorm_d.ap()[blk, :]
                rb_src = bass.AP(
                    tensor=base.tensor,
                    offset=base.offset,
                    ap=[[0, 64]] + [list(p) for p in base.ap],
                )
                nc.sync.dma_start(out=rb[:], in_=rb_src)
                outn = evac_pool.tile([64, 2 * TB], BF16, tag="on")
                with nc.allow_low_precision(reason="normalized attn out as bf16"):
                    nc.vector.tensor_mul(outn[:], o_un[:], rb[:])
                chunk = b * 2 + ib // 2
                nc.sync.dma_start(out=a2i[half].ap()[chunk, 0:64, :], in_=outn[:, 0:TB])
                nc.sync.dma_start(out=a2i[half].ap()[chunk, 64:128, :], in_=outn[:, TB:2 * TB])

            for b in range(B):
                for ib in (0, 2):
                    attn_block(b, ib, 0)
            nc.gpsimd.collective_compute(
                "AllToAll", mybir.AluOpType.bypass,
                ins=[a2i[0].ap()], outs=[a2o[0].ap()],
                replica_groups=[list(range(N_CORES))],
            )
            wproj_sb = wp_pool.tile([128, NKT, C], BF16)
            nc.sync.dma_start(out=wproj_sb, in_=wprojT.ap().rearrange("(kt p) m -> p kt m", p=128))

            for b in range(B):
                for ib in (1, 3):
                    attn_block(b, ib, 1)

            # ---- output projection for my 1024-token slice ----
            for half in range(2):
                if half == 1:
                    nc.gpsimd.collective_compute(
                        "AllToAll", mybir.AluOpType.bypass,
                        ins=[a2i[1].ap()], outs=[a2o[1].ap()],
                        replica_groups=[list(range(N_CORES))],
                    )
                at = xt_pool.tile([128, NKT, TB], BF16, tag="xt")
                for kt in range(NKT):
                    nc.gpsimd.dma_start(
                        out=at[:, kt, :],
                        in_=a2o[half].ap()[kt, :, :],
                    )
                for dt in range(8):
                    ps = ps_a.tile([128, TB], F32, tag="a")
                    for kt in range(NKT):
                        nc.tensor.matmul(
                            ps[:],
                            lhsT=wproj_sb[:, kt, dt * 128:(dt + 1) * 128],
                            rhs=at[:, kt, :],
                            start=(kt == 0),
                            stop=(kt == NKT - 1),
                        )
                    ot = po_pool.tile([128, TB], F32, tag="po")
                    nc.scalar.activation(
                        ot[:], ps[:], mybir.ActivationFunctionType.Identity,
                        bias=bmat_sb[:, dt:dt + 1], scale=1.0,
                    )
                    nc.sync.dma_start(
                        out=outT.ap()[dt * 128:(dt + 1) * 128, half * TB:(half + 1) * TB],
                        in_=ot[:],
                    )

    nc.compile()
    return nc


_NC = None
_last_in_maps = None


def _get_nc():
    global _NC
    if _NC is None:
        _NC = build()
    return _NC


def kernel(x, w_qkv, w_proj, b_proj):
    nc = _get_nc()

    x = np.asarray(x, dtype=np.float32)
    w_qkv = np.asarray(w_qkv, dtype=np.float32)
    w_proj = np.asarray(w_proj, dtype=np.float32)
    b_proj = np.asarray(b_proj, dtype=np.float32)

    xT = np.ascontiguousarray(x.reshape(BT, C).T)
    wprojT = np.ascontiguousarray(w_proj.T).astype(ml_dtypes.bfloat16)
    bmat = np.ascontiguousarray(b_proj.reshape(8, 128).T)
    p = np.arange(128)[:, None]
    f = np.arange(TB)[None, :]
    mask1 = np.stack([(k * 128 + p <= f) for k in range(4)], axis=1)  # [128, 4, 512]
    masks_np = np.concatenate([mask1, mask1], axis=2).astype(ml_dtypes.bfloat16)
    ident = np.eye(128, dtype=np.float32).astype(ml_dtypes.bfloat16)
    onescol = np.ones((128, 1), dtype=np.float32).astype(ml_dtypes.bfloat16)

    in_maps = []
    for c in range(N_CORES):
        rows = slice(c * 128, (c + 1) * 128)
        w_local = np.concatenate(
            [w_qkv[0:C][rows], w_qkv[C:2 * C][rows], w_qkv[2 * C:3 * C][rows]], axis=0
        )  # [384, C]
        in_maps.append({
            "xT": xT,
            "wqkvT": np.ascontiguousarray(w_local.T),
            "wprojT": wprojT,
            "bmat": bmat,
            "masks": masks_np,
            "ident": ident,
            "onescol": onescol,
        })

    global _last_in_maps
    _last_in_maps = in_maps
    res = run_bass_kernel_spmd(nc, in_maps, core_ids=list(range(N_CORES)))
    outT_full = np.concatenate([res.results[c]["outT"] for c in range(N_CORES)], axis=1)
    return np.ascontiguousarray(outT_full.T).reshape(B, T, C)
